# revision 38
# baseline (speedup 1.0000x reference)
"""CapsuleLayer (dynamic routing, 3 iterations) Trainium2 Bass kernel. V2.

Problem (hardcoded):
    x: [64, 2048, 8] f32, W: [2048, 32, 8, 16] f32
    u_hat[b,o,i,k] = sum_d x[b,i,d] * W[i,o,d,k]
    3 rounds of routing-by-agreement over logits b[B,O,I], softmax over O.
    out v: [64, 32, 16] f32.

Sharding: data-parallel over batch across 8 cores (8 batch rows each), W
replicated. Per-core layout: partitions = (g, b) with i = t*16 + g,
free columns = (k, o): col = k*32 + o.

V2 engine plan (vs V1): batch the softmax per 8-tile group (1 big exp on
ACT instead of per-tile exp+accum), z-reduce + tree mostly on DVE, tail
tree levels + logit update + a slice of the cu-mul on GPSIMD, per-tile
c=e*rz scaling on ACT, PSUM->SBUF u16 copies split ACT/DVE/GPSIMD,
W/xblk DMA batched into partition-major chunks issued from SP, and the
v-broadcast consumed via stride-0 AP (no vexp replication).
"""

import numpy as np
import ml_dtypes

BF16 = ml_dtypes.bfloat16
FP8 = ml_dtypes.float8_e4m3fn

B, I, D, O, K = 64, 2048, 8, 32, 16
NC_N = 8              # cores
BL = B // NC_N        # 8 batch rows per core
G = 16                # i's per tile
T = I // G            # 128 tiles
FREE = O * K          # 512, layout (k,o): col = k*32+o
EPS = 1e-7
BATCH = 8             # tiles per instruction group in the rounds
NGRP = T // BATCH     # 16 groups
WCH = 8               # W tiles per DMA chunk

# pass-0 PSUM->SBUF copy engines (GPSIMD cannot touch PSUM):
# A=ACT ~585ns, D=DVE ~658ns per tile
COPY_PAT = "ADADADADADADADAD"

_CACHE = {}


def _build_bass():
    import concourse.bass as bass
    import concourse.bacc as bacc
    import concourse.mybir as mybir
    import concourse.tile as tile

    f32 = mybir.dt.float32
    bf16 = mybir.dt.bfloat16
    f8 = mybir.dt.float8e4
    nc = bacc.Bacc()

    # DRAM tensors (partition-major for cheap descriptors)
    wd = nc.dram_tensor("w", [128, T, FREE], bf16, kind="ExternalInput")
    xtd = nc.dram_tensor("xt", [128, T, BL], bf16, kind="ExternalInput")
    xblkd = nc.dram_tensor("xblk", [128, T, 128], bf16, kind="ExternalInput")
    onesd = nc.dram_tensor("ones", [128, BL], bf16, kind="ExternalInput")
    onestd = nc.dram_tensor("onest", [BL, 128], bf16, kind="ExternalInput")
    outd = nc.dram_tensor("out", [BL, FREE], f32, kind="ExternalOutput")

    AX = mybir.AxisListType
    ALU = mybir.AluOpType
    ACTF = mybir.ActivationFunctionType

    with tile.TileContext(nc) as tc:
        with (
            tc.tile_pool(name="const", bufs=1) as constp,
            tc.tile_pool(name="u16", bufs=1) as up,
            tc.tile_pool(name="logits", bufs=1) as blp,
            tc.tile_pool(name="vexp", bufs=2) as vexpp,
            tc.tile_pool(name="psum_s", bufs=2, space="PSUM") as psum_s,
            tc.tile_pool(name="psum_v", bufs=1, space="PSUM") as psum_v,
        ):
            eps_sb = constp.tile([128, 1], f32)
            nc.gpsimd.memset(eps_sb[:], EPS)
            xt_sb = constp.tile([128, T, BL], bf16)
            ones_sb = constp.tile([128, BL], bf16)
            onest_sb = constp.tile([BL, 128], bf16)

            u16 = up.tile([128, T, FREE], bf16)
            bL = blp.tile([128, T, O], bf16)

            # ---------------- pass 0: u_hat + s0 ----------------
            s0_ps = psum_s.tile([BL, FREE], f32)
            with (
                tc.tile_pool(name="xblk", bufs=4) as xblkp,
                tc.tile_pool(name="wt", bufs=5) as wtp,
                tc.tile_pool(name="psum_u", bufs=3, space="PSUM") as psum_u,
            ):
                NCH = T // WCH
                DELAY = 3  # chunks by which u_hat emission trails s0
                wts, xbs = {}, {}

                def emit_uhat(ci):
                    for j in range(WCH):
                        t = ci * WCH + j
                        ut_ps = psum_u.tile([128, FREE], f32, tag="ut")
                        nc.tensor.matmul(
                            ut_ps[:], xbs[ci][:, j, :], wts[ci][:, j, :])
                        # halves on both engines: ~400ns/tile drain so
                        # the copy drain no longer throttles PE
                        nc.scalar.copy(u16[:, t, 0:256], ut_ps[:, 0:256])
                        nc.vector.tensor_copy(
                            u16[:, t, 256:512], ut_ps[:, 256:512])

                for ci in range(NCH):
                    tc0 = ci * WCH
                    # chunked, partition-major loads: 128 descs per issue
                    wt = wtp.tile([128, WCH, FREE], bf16, tag="wt")
                    nc.sync.dma_start(wt[:], wd[:, tc0:tc0 + WCH, :])
                    xb = xblkp.tile([128, WCH, 128], bf16, tag="xb")
                    nc.sync.dma_start(xb[:], xblkd[:, tc0:tc0 + WCH, :])
                    wts[ci], xbs[ci] = wt, xb
                    if ci == 0:
                        # consts issued after chunk 0 so its transfer and
                        # the first matmuls start as early as possible
                        nc.sync.dma_start(xt_sb[:], xtd[:])
                        nc.sync.dma_start(ones_sb[:], onesd[:])
                        nc.sync.dma_start(onest_sb[:], onestd[:])
                    # s0 matmuls lead; u_hat matmuls trail by DELAY chunks
                    # so s0 (which gates round 1) completes sooner and the
                    # u_hat tail overlaps round 1
                    for j in range(WCH):
                        t = tc0 + j
                        nc.tensor.matmul(
                            s0_ps[:], xt_sb[:, t, :], wt[:, j, :],
                            start=(t == 0), stop=(t == T - 1),
                        )
                    if ci >= DELAY:
                        emit_uhat(ci - DELAY)
                for ci in range(NCH - DELAY, NCH):
                    emit_uhat(ci)

            # ---------------- squash + broadcast helper ----------------
            with tc.tile_pool(name="sq", bufs=1) as sqp:

                def squash_and_bcast(s_ps, scale_const, last):
                    """v = squash(s_ps * scale_const). Returns vexp1
                    [128, FREE] bf16 (v replicated to all partition groups)
                    or DMAs fp32 v to outd if last."""
                    s = sqp.tile([BL, FREE], f32, tag="s")
                    nc.scalar.mul(s[:], s_ps[:], scale_const)
                    # s2[o] = sum_k s^2  (k stride is O in (k,o) layout)
                    sq2 = sqp.tile([BL, O, K], f32, tag="sq2")
                    nc.vector.tensor_mul(
                        sq2[:], s[:].rearrange("p (k o) -> p o k", o=O),
                        s[:].rearrange("p (k o) -> p o k", o=O))
                    s2 = sqp.tile([BL, O], f32, tag="s2")
                    nc.vector.reduce_sum(s2[:], sq2[:], axis=AX.X)
                    rt = sqp.tile([BL, O], f32, tag="rt")
                    nc.scalar.activation(rt[:], s2[:], ACTF.Sqrt, bias=eps_sb[:BL])
                    onep = sqp.tile([BL, O], f32, tag="onep")
                    nc.scalar.add(onep[:], s2[:], 1.0)
                    den = sqp.tile([BL, O], f32, tag="den")
                    nc.vector.tensor_mul(den[:], rt[:], onep[:])
                    rden = sqp.tile([BL, O], f32, tag="rden")
                    nc.vector.reciprocal(rden[:], den[:])
                    scl = sqp.tile([BL, O], f32, tag="scl")
                    nc.vector.tensor_mul(scl[:], s2[:], rden[:])
                    # v = s * scl (broadcast over k)
                    v = sqp.tile([BL, K, O], f32 if last else bf16, tag="v")
                    nc.vector.tensor_mul(
                        v[:], s[:].rearrange("p (k o) -> p k o", o=O),
                        scl[:].unsqueeze(1).broadcast_to([BL, K, O]))
                    if last:
                        nc.gpsimd.dma_start(outd[:], v[:].rearrange("p k o -> p (k o)"))
                        return None
                    # replicate v to all 16 partition groups via PE
                    vrep_ps = psum_v.tile([128, FREE], f32, tag="vrep")
                    nc.tensor.matmul(
                        vrep_ps[:], onest_sb[:],
                        v[:].rearrange("p k o -> p (k o)"))
                    vexp1 = vexpp.tile([128, FREE], bf16, tag="vexp1")
                    nc.scalar.copy(vexp1[:], vrep_ps[:])
                    return vexp1

                vexp1 = squash_and_bcast(s0_ps, 1.0 / O, last=False)

                # ---------------- rounds 1, 2 ----------------
                with (
                    tc.tile_pool(name="rnd", bufs=2) as rp,
                    tc.tile_pool(name="rnd2", bufs=3) as rp2,
                    tc.tile_pool(name="vex8", bufs=1) as v8p,
                ):
                    for rnd in (1, 2):
                        # replicate v over the t axis once per round so the
                        # hot vu-mul is a plain tensor-tensor op (2x DVE mode)
                        vexp8 = v8p.tile([128, BATCH, FREE], bf16, tag="vexp8")
                        for j in range(BATCH):
                            nc.vector.tensor_copy(vexp8[:, j, :], vexp1[:])
                        s_ps = psum_s.tile([BL, FREE], f32, tag="s_ps")

                        # Software-pipelined over groups with a 2-group skew:
                        # engines run their queues in order, so emitting
                        # rz/c/cu right after exp would head-of-line block
                        # DVE on ACT. front = agreement + exp, mid = rz + c,
                        # back = cu + PE accumulation.
                        state = {}

                        def front(g):
                            tb = g * BATCH
                            u_sl = u16[:, tb:tb + BATCH, :]
                            vu = rp.tile([128, BATCH, FREE], bf16, tag="vu")
                            nc.vector.tensor_mul(vu[:], u_sl, vexp8[:])
                            # k-tree: in (k,o) layout the k halves are
                            # contiguous column blocks, all ops stay 3-D
                            t1 = rp.tile([128, BATCH, FREE // 2], bf16, tag="t1")
                            nc.vector.tensor_add(
                                t1[:], vu[:, :, 0:256], vu[:, :, 256:512])
                            t2 = rp.tile([128, BATCH, FREE // 4], bf16, tag="t2")
                            nc.vector.tensor_add(
                                t2[:], t1[:, :, 0:128], t1[:, :, 128:256])
                            t3 = rp2.tile([128, BATCH, FREE // 8], bf16, tag="t3")
                            lgs = bL[:, tb:tb + BATCH, :]
                            nc.vector.tensor_add(
                                t3[:], t2[:, :, 0:64], t2[:, :, 64:128])
                            if rnd == 1:
                                # b1 = agreement (b0 == 0)
                                nc.vector.tensor_add(
                                    lgs, t3[:, :, 0:32], t3[:, :, 32:64])
                            else:
                                agr = rp2.tile([128, BATCH, O], bf16, tag="agr")
                                nc.vector.tensor_add(
                                    agr[:], t3[:, :, 0:32], t3[:, :, 32:64])
                                # b2 = b1 + agreement (in place)
                                nc.vector.tensor_add(lgs, agr[:], lgs)
                            # per-tile exp on ACT, accumulator supplies z
                            e = rp2.tile([128, BATCH, O], bf16, tag="e")
                            z = rp2.tile([128, BATCH], f32, tag="z")
                            for j in range(BATCH):
                                nc.scalar.activation(
                                    e[:, j, :], lgs[:, j, :], ACTF.Exp,
                                    accum_out=z[:, j:j + 1])
                            state[g] = (u_sl, e, z)

                        def mid(g):
                            u_sl, e, z = state[g]
                            rz = rp2.tile([128, BATCH], f32, tag="rz")
                            nc.vector.reciprocal(rz[:], z[:])
                            c = rp2.tile([128, BATCH, O], bf16, tag="c")
                            for j in range(BATCH):
                                nc.scalar.activation(
                                    c[:, j, :], e[:, j, :], ACTF.Copy,
                                    scale=rz[:, j:j + 1])
                            state[g] = (u_sl, c)

                        def back(g):
                            u_sl, c = state.pop(g)
                            cu = rp.tile([128, BATCH, K, O], bf16, tag="cu")
                            for j in range(BATCH):
                                nc.vector.tensor_mul(
                                    cu[:, j],
                                    u_sl[:, j].rearrange("p (k o) -> p k o", o=O),
                                    c[:, j].unsqueeze(1).broadcast_to(
                                        [128, K, O]))
                            # s += sum_i cu  (PE partition reduce via ones)
                            for j in range(BATCH):
                                t = g * BATCH + j
                                nc.tensor.matmul(
                                    s_ps[:], ones_sb[:],
                                    cu[:, j, :, :].rearrange("p k o -> p (k o)"),
                                    start=(t == 0), stop=(t == T - 1))

                        for g in range(NGRP + 2):
                            if g < NGRP:
                                front(g)
                            if 1 <= g < NGRP + 1:
                                mid(g - 1)
                            if g >= 2:
                                back(g - 2)
                        vexp1 = squash_and_bcast(s_ps, 1.0, last=(rnd == 2))
    nc.finalize()
    return nc


def _host_prep():
    """Core-independent input prep pieces."""
    ones = np.zeros((128, BL), dtype=BF16)
    for g in range(G):
        for b in range(BL):
            ones[g * 8 + b, b] = 1
    onest = np.ascontiguousarray(ones.T)
    return ones, onest


def kernel(x: np.ndarray, W: np.ndarray) -> np.ndarray:
    from concourse import bass_utils

    if "nc" not in _CACHE:
        _CACHE["nc"] = _build_bass()
        _CACHE["ones"], _CACHE["onest"] = _host_prep()
    nc = _CACHE["nc"]

    # W -> [(g,d), t, (k,o)] : w[g*8+d, t, k*32+o] = W[t*16+g, o, d, k]
    wr = np.ascontiguousarray(
        (W.reshape(T, G, O, D, K).transpose(0, 1, 3, 4, 2)
         .reshape(T, 128, FREE)).transpose(1, 0, 2)).astype(BF16)
    in_maps = []
    for c in range(NC_N):
        xl = x[c * BL:(c + 1) * BL]  # [8, 2048, 8]
        # xt[g*8+d, t, b] = xl[b, t*16+g, d]
        xt = np.ascontiguousarray(
            xl.reshape(BL, T, G, D).transpose(2, 3, 1, 0).reshape(128, T, BL)
        ).astype(BF16)
        xblk = np.zeros((128, T, 128), dtype=BF16)
        for g in range(G):
            xblk[g * 8:(g + 1) * 8, :, g * 8:(g + 1) * 8] = xt[g * 8:(g + 1) * 8]
        in_maps.append({"w": wr, "xt": xt, "xblk": xblk, "ones": _CACHE["ones"],
                        "onest": _CACHE["onest"]})

    _CACHE["in_maps"] = in_maps
    res = bass_utils.run_bass_kernel_spmd(nc, in_maps, core_ids=list(range(NC_N)))
    out = np.empty((B, O, K), np.float32)
    for c in range(NC_N):
        v = res.results[c]["out"].reshape(BL, K, O)  # (k,o) cols
        out[c * BL:(c + 1) * BL] = v.transpose(0, 2, 1)
    return out


# revision 42
# speedup vs baseline: 1.0148x; 1.0148x over previous
"""CapsuleLayer (dynamic routing, 3 iterations) Trainium2 Bass kernel. V2.

Problem (hardcoded):
    x: [64, 2048, 8] f32, W: [2048, 32, 8, 16] f32
    u_hat[b,o,i,k] = sum_d x[b,i,d] * W[i,o,d,k]
    3 rounds of routing-by-agreement over logits b[B,O,I], softmax over O.
    out v: [64, 32, 16] f32.

Sharding: data-parallel over batch across 8 cores (8 batch rows each), W
replicated. Per-core layout: partitions = (g, b) with i = t*16 + g,
free columns = (k, o): col = k*32 + o.

V2 engine plan (vs V1): batch the softmax per 8-tile group (1 big exp on
ACT instead of per-tile exp+accum), z-reduce + tree mostly on DVE, tail
tree levels + logit update + a slice of the cu-mul on GPSIMD, per-tile
c=e*rz scaling on ACT, PSUM->SBUF u16 copies split ACT/DVE/GPSIMD,
W/xblk DMA batched into partition-major chunks issued from SP, and the
v-broadcast consumed via stride-0 AP (no vexp replication).
"""

import numpy as np
import ml_dtypes

BF16 = ml_dtypes.bfloat16
FP8 = ml_dtypes.float8_e4m3fn

B, I, D, O, K = 64, 2048, 8, 32, 16
NC_N = 8              # cores
BL = B // NC_N        # 8 batch rows per core
G = 16                # i's per tile
T = I // G            # 128 tiles
FREE = O * K          # 512, layout (k,o): col = k*32+o
EPS = 1e-7
BATCH = 8             # tiles per instruction group in the rounds
NGRP = T // BATCH     # 16 groups
WCH = 8               # W tiles per DMA chunk

# pass-0 PSUM->SBUF copy engines (GPSIMD cannot touch PSUM):
# A=ACT ~585ns, D=DVE ~658ns per tile
COPY_PAT = "ADADADADADADADAA"

_CACHE = {}


def _build_bass():
    import concourse.bass as bass
    import concourse.bacc as bacc
    import concourse.mybir as mybir
    import concourse.tile as tile

    f32 = mybir.dt.float32
    bf16 = mybir.dt.bfloat16
    f8 = mybir.dt.float8e4
    nc = bacc.Bacc()

    # DRAM tensors (partition-major for cheap descriptors)
    wd = nc.dram_tensor("w", [128, T, FREE], bf16, kind="ExternalInput")
    xtd = nc.dram_tensor("xt", [128, T, BL], bf16, kind="ExternalInput")
    xblkd = nc.dram_tensor("xblk", [128, T, 128], bf16, kind="ExternalInput")
    onesd = nc.dram_tensor("ones", [128, BL], bf16, kind="ExternalInput")
    onestd = nc.dram_tensor("onest", [BL, 128], bf16, kind="ExternalInput")
    outd = nc.dram_tensor("out", [BL, FREE], f32, kind="ExternalOutput")

    AX = mybir.AxisListType
    ALU = mybir.AluOpType
    ACTF = mybir.ActivationFunctionType

    with tile.TileContext(nc) as tc:
        with (
            tc.tile_pool(name="const", bufs=1) as constp,
            tc.tile_pool(name="u16", bufs=1) as up,
            tc.tile_pool(name="logits", bufs=1) as blp,
            tc.tile_pool(name="vexp", bufs=2) as vexpp,
            tc.tile_pool(name="psum_s", bufs=2, space="PSUM") as psum_s,
            tc.tile_pool(name="psum_v", bufs=1, space="PSUM") as psum_v,
        ):
            eps_sb = constp.tile([128, 1], f32)
            nc.gpsimd.memset(eps_sb[:], EPS)
            xt_sb = constp.tile([128, T, BL], bf16)
            ones_sb = constp.tile([128, BL], bf16)
            onest_sb = constp.tile([BL, 128], bf16)

            u16 = up.tile([128, T, FREE], bf16)
            bL = blp.tile([128, T, O], bf16)

            # ---------------- pass 0: u_hat + s0 ----------------
            s0_ps = psum_s.tile([BL, FREE], f32)
            with (
                tc.tile_pool(name="xblk", bufs=4) as xblkp,
                tc.tile_pool(name="wt", bufs=5) as wtp,
                tc.tile_pool(name="psum_u", bufs=3, space="PSUM") as psum_u,
            ):
                NCH = T // WCH
                DELAY = 3  # chunks by which u_hat emission trails s0
                wts, xbs = {}, {}

                def emit_uhat(ci):
                    for j in range(WCH):
                        t = ci * WCH + j
                        ut_ps = psum_u.tile([128, FREE], f32, tag="ut")
                        nc.tensor.matmul(
                            ut_ps[:], xbs[ci][:, j, :], wts[ci][:, j, :])
                        if COPY_PAT[t % len(COPY_PAT)] == "A":
                            nc.scalar.copy(u16[:, t, :], ut_ps[:])
                        else:
                            nc.vector.tensor_copy(u16[:, t, :], ut_ps[:])

                for ci in range(NCH):
                    tc0 = ci * WCH
                    # chunked, partition-major loads: 128 descs per issue
                    wt = wtp.tile([128, WCH, FREE], bf16, tag="wt")
                    nc.sync.dma_start(wt[:], wd[:, tc0:tc0 + WCH, :])
                    xb = xblkp.tile([128, WCH, 128], bf16, tag="xb")
                    nc.sync.dma_start(xb[:], xblkd[:, tc0:tc0 + WCH, :])
                    wts[ci], xbs[ci] = wt, xb
                    if ci == 0:
                        # consts issued after chunk 0 so its transfer and
                        # the first matmuls start as early as possible
                        nc.sync.dma_start(xt_sb[:], xtd[:])
                        nc.sync.dma_start(ones_sb[:], onesd[:])
                        nc.sync.dma_start(onest_sb[:], onestd[:])
                    # s0 matmuls lead; u_hat matmuls trail by DELAY chunks
                    # so s0 (which gates round 1) completes sooner and the
                    # u_hat tail overlaps round 1
                    for j in range(WCH):
                        t = tc0 + j
                        nc.tensor.matmul(
                            s0_ps[:], xt_sb[:, t, :], wt[:, j, :],
                            start=(t == 0), stop=(t == T - 1),
                        )
                    if ci >= DELAY:
                        emit_uhat(ci - DELAY)
                for ci in range(NCH - DELAY, NCH):
                    emit_uhat(ci)

            # ---------------- squash + broadcast helper ----------------
            with tc.tile_pool(name="sq", bufs=1) as sqp:

                def squash_and_bcast(s_ps, scale_const, last):
                    """v = squash(s_ps * scale_const). Returns vexp1
                    [128, FREE] bf16 (v replicated to all partition groups)
                    or DMAs fp32 v to outd if last."""
                    s = sqp.tile([BL, FREE], f32, tag="s")
                    nc.scalar.mul(s[:], s_ps[:], scale_const)
                    # s2[o] = sum_k s^2  (k stride is O in (k,o) layout)
                    sq2 = sqp.tile([BL, O, K], f32, tag="sq2")
                    nc.vector.tensor_mul(
                        sq2[:], s[:].rearrange("p (k o) -> p o k", o=O),
                        s[:].rearrange("p (k o) -> p o k", o=O))
                    s2 = sqp.tile([BL, O], f32, tag="s2")
                    nc.vector.reduce_sum(s2[:], sq2[:], axis=AX.X)
                    rt = sqp.tile([BL, O], f32, tag="rt")
                    nc.scalar.activation(rt[:], s2[:], ACTF.Sqrt, bias=eps_sb[:BL])
                    onep = sqp.tile([BL, O], f32, tag="onep")
                    nc.scalar.add(onep[:], s2[:], 1.0)
                    den = sqp.tile([BL, O], f32, tag="den")
                    nc.vector.tensor_mul(den[:], rt[:], onep[:])
                    rden = sqp.tile([BL, O], f32, tag="rden")
                    nc.vector.reciprocal(rden[:], den[:])
                    scl = sqp.tile([BL, O], f32, tag="scl")
                    nc.vector.tensor_mul(scl[:], s2[:], rden[:])
                    # v = s * scl (broadcast over k)
                    v = sqp.tile([BL, K, O], f32 if last else bf16, tag="v")
                    nc.vector.tensor_mul(
                        v[:], s[:].rearrange("p (k o) -> p k o", o=O),
                        scl[:].unsqueeze(1).broadcast_to([BL, K, O]))
                    if last:
                        nc.gpsimd.dma_start(outd[:], v[:].rearrange("p k o -> p (k o)"))
                        return None
                    # replicate v to all 16 partition groups via PE
                    vrep_ps = psum_v.tile([128, FREE], f32, tag="vrep")
                    nc.tensor.matmul(
                        vrep_ps[:], onest_sb[:],
                        v[:].rearrange("p k o -> p (k o)"))
                    vexp1 = vexpp.tile([128, FREE], bf16, tag="vexp1")
                    nc.scalar.copy(vexp1[:], vrep_ps[:])
                    return vexp1

                vexp1 = squash_and_bcast(s0_ps, 1.0 / O, last=False)

                # ---------------- rounds 1, 2 ----------------
                with (
                    tc.tile_pool(name="rnd", bufs=2) as rp,
                    tc.tile_pool(name="rnd2", bufs=3) as rp2,
                    tc.tile_pool(name="vex8", bufs=1) as v8p,
                ):
                    for rnd in (1, 2):
                        # replicate v over the t axis once per round so the
                        # hot vu-mul is a plain tensor-tensor op (2x DVE mode)
                        vexp8 = v8p.tile([128, BATCH, FREE], bf16, tag="vexp8")
                        for j in range(BATCH):
                            nc.vector.tensor_copy(vexp8[:, j, :], vexp1[:])
                        s_ps = psum_s.tile([BL, FREE], f32, tag="s_ps")

                        # Software-pipelined over groups with a 2-group skew:
                        # engines run their queues in order, so emitting
                        # rz/c/cu right after exp would head-of-line block
                        # DVE on ACT. front = agreement + exp, mid = rz + c,
                        # back = cu + PE accumulation.
                        state = {}

                        def front(g):
                            tb = g * BATCH
                            u_sl = u16[:, tb:tb + BATCH, :]
                            vu = rp.tile([128, BATCH, FREE], bf16, tag="vu")
                            nc.vector.tensor_mul(vu[:], u_sl, vexp8[:])
                            # k-tree: in (k,o) layout the k halves are
                            # contiguous column blocks, all ops stay 3-D
                            t1 = rp.tile([128, BATCH, FREE // 2], bf16, tag="t1")
                            nc.vector.tensor_add(
                                t1[:], vu[:, :, 0:256], vu[:, :, 256:512])
                            t2 = rp.tile([128, BATCH, FREE // 4], bf16, tag="t2")
                            nc.vector.tensor_add(
                                t2[:], t1[:, :, 0:128], t1[:, :, 128:256])
                            t3 = rp2.tile([128, BATCH, FREE // 8], bf16, tag="t3")
                            lgs = bL[:, tb:tb + BATCH, :]
                            nc.vector.tensor_add(
                                t3[:], t2[:, :, 0:64], t2[:, :, 64:128])
                            if rnd == 1:
                                # b1 = agreement (b0 == 0)
                                nc.vector.tensor_add(
                                    lgs, t3[:, :, 0:32], t3[:, :, 32:64])
                            else:
                                agr = rp2.tile([128, BATCH, O], bf16, tag="agr")
                                nc.vector.tensor_add(
                                    agr[:], t3[:, :, 0:32], t3[:, :, 32:64])
                                # b2 = b1 + agreement (in place)
                                nc.vector.tensor_add(lgs, agr[:], lgs)
                            # per-tile exp on ACT, accumulator supplies z
                            e = rp2.tile([128, BATCH, O], bf16, tag="e")
                            z = rp2.tile([128, BATCH], f32, tag="z")
                            for j in range(BATCH):
                                nc.scalar.activation(
                                    e[:, j, :], lgs[:, j, :], ACTF.Exp,
                                    accum_out=z[:, j:j + 1])
                            state[g] = (u_sl, e, z)

                        def mid(g):
                            u_sl, e, z = state[g]
                            rz = rp2.tile([128, BATCH], f32, tag="rz")
                            nc.vector.reciprocal(rz[:], z[:])
                            c = rp2.tile([128, BATCH, O], bf16, tag="c")
                            for j in range(BATCH):
                                nc.scalar.activation(
                                    c[:, j, :], e[:, j, :], ACTF.Copy,
                                    scale=rz[:, j:j + 1])
                            state[g] = (u_sl, c)

                        def back(g):
                            u_sl, c = state.pop(g)
                            cu = rp.tile([128, BATCH, K, O], bf16, tag="cu")
                            for j in range(BATCH):
                                nc.vector.tensor_mul(
                                    cu[:, j],
                                    u_sl[:, j].rearrange("p (k o) -> p k o", o=O),
                                    c[:, j].unsqueeze(1).broadcast_to(
                                        [128, K, O]))
                            # s += sum_i cu  (PE partition reduce via ones)
                            for j in range(BATCH):
                                t = g * BATCH + j
                                nc.tensor.matmul(
                                    s_ps[:], ones_sb[:],
                                    cu[:, j, :, :].rearrange("p k o -> p (k o)"),
                                    start=(t == 0), stop=(t == T - 1))

                        for g in range(NGRP + 2):
                            if g < NGRP:
                                front(g)
                            if 1 <= g < NGRP + 1:
                                mid(g - 1)
                            if g >= 2:
                                back(g - 2)
                        vexp1 = squash_and_bcast(s_ps, 1.0, last=(rnd == 2))
    nc.finalize()
    return nc


def _host_prep():
    """Core-independent input prep pieces."""
    ones = np.zeros((128, BL), dtype=BF16)
    for g in range(G):
        for b in range(BL):
            ones[g * 8 + b, b] = 1
    onest = np.ascontiguousarray(ones.T)
    return ones, onest


def kernel(x: np.ndarray, W: np.ndarray) -> np.ndarray:
    from concourse import bass_utils

    if "nc" not in _CACHE:
        _CACHE["nc"] = _build_bass()
        _CACHE["ones"], _CACHE["onest"] = _host_prep()
    nc = _CACHE["nc"]

    # W -> [(g,d), t, (k,o)] : w[g*8+d, t, k*32+o] = W[t*16+g, o, d, k]
    wr = np.ascontiguousarray(
        (W.reshape(T, G, O, D, K).transpose(0, 1, 3, 4, 2)
         .reshape(T, 128, FREE)).transpose(1, 0, 2)).astype(BF16)
    in_maps = []
    for c in range(NC_N):
        xl = x[c * BL:(c + 1) * BL]  # [8, 2048, 8]
        # xt[g*8+d, t, b] = xl[b, t*16+g, d]
        xt = np.ascontiguousarray(
            xl.reshape(BL, T, G, D).transpose(2, 3, 1, 0).reshape(128, T, BL)
        ).astype(BF16)
        xblk = np.zeros((128, T, 128), dtype=BF16)
        for g in range(G):
            xblk[g * 8:(g + 1) * 8, :, g * 8:(g + 1) * 8] = xt[g * 8:(g + 1) * 8]
        in_maps.append({"w": wr, "xt": xt, "xblk": xblk, "ones": _CACHE["ones"],
                        "onest": _CACHE["onest"]})

    _CACHE["in_maps"] = in_maps
    res = bass_utils.run_bass_kernel_spmd(nc, in_maps, core_ids=list(range(NC_N)))
    out = np.empty((B, O, K), np.float32)
    for c in range(NC_N):
        v = res.results[c]["out"].reshape(BL, K, O)  # (k,o) cols
        out[c * BL:(c + 1) * BL] = v.transpose(0, 2, 1)
    return out


# revision 43
# speedup vs baseline: 1.0195x; 1.0047x over previous
"""CapsuleLayer (dynamic routing, 3 iterations) Trainium2 Bass kernel. V2.

Problem (hardcoded):
    x: [64, 2048, 8] f32, W: [2048, 32, 8, 16] f32
    u_hat[b,o,i,k] = sum_d x[b,i,d] * W[i,o,d,k]
    3 rounds of routing-by-agreement over logits b[B,O,I], softmax over O.
    out v: [64, 32, 16] f32.

Sharding: data-parallel over batch across 8 cores (8 batch rows each), W
replicated. Per-core layout: partitions = (g, b) with i = t*16 + g,
free columns = (k, o): col = k*32 + o.

V2 engine plan (vs V1): batch the softmax per 8-tile group (1 big exp on
ACT instead of per-tile exp+accum), z-reduce + tree mostly on DVE, tail
tree levels + logit update + a slice of the cu-mul on GPSIMD, per-tile
c=e*rz scaling on ACT, PSUM->SBUF u16 copies split ACT/DVE/GPSIMD,
W/xblk DMA batched into partition-major chunks issued from SP, and the
v-broadcast consumed via stride-0 AP (no vexp replication).
"""

import numpy as np
import ml_dtypes

BF16 = ml_dtypes.bfloat16
FP8 = ml_dtypes.float8_e4m3fn

B, I, D, O, K = 64, 2048, 8, 32, 16
NC_N = 8              # cores
BL = B // NC_N        # 8 batch rows per core
G = 16                # i's per tile
T = I // G            # 128 tiles
FREE = O * K          # 512, layout (k,o): col = k*32+o
EPS = 1e-7
BATCH = 8             # tiles per instruction group in the rounds
NGRP = T // BATCH     # 16 groups
WCH = 8               # W tiles per DMA chunk

# pass-0 PSUM->SBUF copy engines (GPSIMD cannot touch PSUM):
# A=ACT ~585ns, D=DVE ~658ns per tile
COPY_PAT = "ADADADADADADADAD"

_CACHE = {}


def _build_bass():
    import concourse.bass as bass
    import concourse.bacc as bacc
    import concourse.mybir as mybir
    import concourse.tile as tile

    f32 = mybir.dt.float32
    bf16 = mybir.dt.bfloat16
    f8 = mybir.dt.float8e4
    nc = bacc.Bacc()

    # DRAM tensors (partition-major for cheap descriptors)
    wd = nc.dram_tensor("w", [128, T, FREE], bf16, kind="ExternalInput")
    xtd = nc.dram_tensor("xt", [128, T, BL], bf16, kind="ExternalInput")
    xblkd = nc.dram_tensor("xblk", [128, T, 128], bf16, kind="ExternalInput")
    onesd = nc.dram_tensor("ones", [128, BL], bf16, kind="ExternalInput")
    onestd = nc.dram_tensor("onest", [BL, 128], bf16, kind="ExternalInput")
    outd = nc.dram_tensor("out", [BL, FREE], f32, kind="ExternalOutput")

    AX = mybir.AxisListType
    ALU = mybir.AluOpType
    ACTF = mybir.ActivationFunctionType

    with tile.TileContext(nc) as tc:
        with (
            tc.tile_pool(name="const", bufs=1) as constp,
            tc.tile_pool(name="u16", bufs=1) as up,
            tc.tile_pool(name="logits", bufs=1) as blp,
            tc.tile_pool(name="vexp", bufs=2) as vexpp,
            tc.tile_pool(name="psum_s", bufs=2, space="PSUM") as psum_s,
            tc.tile_pool(name="psum_v", bufs=1, space="PSUM") as psum_v,
        ):
            eps_sb = constp.tile([128, 1], f32)
            nc.gpsimd.memset(eps_sb[:], EPS)
            xt_sb = constp.tile([128, T, BL], bf16)
            ones_sb = constp.tile([128, BL], bf16)
            onest_sb = constp.tile([BL, 128], bf16)

            u16 = up.tile([128, T, FREE], bf16)
            bL = blp.tile([128, T, O], bf16)

            # ---------------- pass 0: u_hat + s0 ----------------
            s0_ps = psum_s.tile([BL, FREE], f32)
            with (
                tc.tile_pool(name="xblk", bufs=4) as xblkp,
                tc.tile_pool(name="wt", bufs=5) as wtp,
                tc.tile_pool(name="psum_u", bufs=3, space="PSUM") as psum_u,
            ):
                NCH = T // WCH
                DELAY = 3  # chunks by which u_hat emission trails s0
                wts, xbs = {}, {}

                def emit_uhat(ci):
                    for j in range(WCH):
                        t = ci * WCH + j
                        ut_ps = psum_u.tile([128, FREE], f32, tag="ut")
                        nc.tensor.matmul(
                            ut_ps[:], xbs[ci][:, j, :], wts[ci][:, j, :])
                        if COPY_PAT[t % len(COPY_PAT)] == "A":
                            nc.scalar.copy(u16[:, t, :], ut_ps[:])
                        else:
                            nc.vector.tensor_copy(u16[:, t, :], ut_ps[:])

                for ci in range(NCH):
                    tc0 = ci * WCH
                    # chunked, partition-major loads: 128 descs per issue
                    wt = wtp.tile([128, WCH, FREE], bf16, tag="wt")
                    nc.sync.dma_start(wt[:], wd[:, tc0:tc0 + WCH, :])
                    xb = xblkp.tile([128, WCH, 128], bf16, tag="xb")
                    nc.sync.dma_start(xb[:], xblkd[:, tc0:tc0 + WCH, :])
                    wts[ci], xbs[ci] = wt, xb
                    if ci == 0:
                        # consts issued after chunk 0 so its transfer and
                        # the first matmuls start as early as possible
                        nc.sync.dma_start(xt_sb[:], xtd[:])
                        nc.sync.dma_start(ones_sb[:], onesd[:])
                        nc.sync.dma_start(onest_sb[:], onestd[:])
                    # s0 matmuls lead; u_hat matmuls trail by DELAY chunks
                    # so s0 (which gates round 1) completes sooner and the
                    # u_hat tail overlaps round 1
                    for j in range(WCH):
                        t = tc0 + j
                        nc.tensor.matmul(
                            s0_ps[:], xt_sb[:, t, :], wt[:, j, :],
                            start=(t == 0), stop=(t == T - 1),
                        )
                    if ci >= DELAY:
                        emit_uhat(ci - DELAY)
                for ci in range(NCH - DELAY, NCH):
                    emit_uhat(ci)

            # ---------------- squash + broadcast helper ----------------
            with tc.tile_pool(name="sq", bufs=1) as sqp:

                def squash_and_bcast(s_ps, scale_const, last):
                    """v = squash(s_ps * scale_const). Returns vexp1
                    [128, FREE] bf16 (v replicated to all partition groups)
                    or DMAs fp32 v to outd if last."""
                    s = sqp.tile([BL, FREE], f32, tag="s")
                    nc.scalar.mul(s[:], s_ps[:], scale_const)
                    # s2[o] = sum_k s^2  (k stride is O in (k,o) layout)
                    sq2 = sqp.tile([BL, O, K], f32, tag="sq2")
                    nc.vector.tensor_mul(
                        sq2[:], s[:].rearrange("p (k o) -> p o k", o=O),
                        s[:].rearrange("p (k o) -> p o k", o=O))
                    s2 = sqp.tile([BL, O], f32, tag="s2")
                    nc.vector.reduce_sum(s2[:], sq2[:], axis=AX.X)
                    rt = sqp.tile([BL, O], f32, tag="rt")
                    nc.scalar.activation(rt[:], s2[:], ACTF.Sqrt, bias=eps_sb[:BL])
                    onep = sqp.tile([BL, O], f32, tag="onep")
                    nc.scalar.add(onep[:], s2[:], 1.0)
                    den = sqp.tile([BL, O], f32, tag="den")
                    nc.vector.tensor_mul(den[:], rt[:], onep[:])
                    rden = sqp.tile([BL, O], f32, tag="rden")
                    nc.vector.reciprocal(rden[:], den[:])
                    scl = sqp.tile([BL, O], f32, tag="scl")
                    nc.vector.tensor_mul(scl[:], s2[:], rden[:])
                    # v = s * scl (broadcast over k)
                    v = sqp.tile([BL, K, O], f32 if last else bf16, tag="v")
                    nc.vector.tensor_mul(
                        v[:], s[:].rearrange("p (k o) -> p k o", o=O),
                        scl[:].unsqueeze(1).broadcast_to([BL, K, O]))
                    if last:
                        nc.gpsimd.dma_start(outd[:], v[:].rearrange("p k o -> p (k o)"))
                        return None
                    # replicate v to all 16 partition groups via PE
                    vrep_ps = psum_v.tile([128, FREE], f32, tag="vrep")
                    nc.tensor.matmul(
                        vrep_ps[:], onest_sb[:],
                        v[:].rearrange("p k o -> p (k o)"))
                    vexp1 = vexpp.tile([128, FREE], bf16, tag="vexp1")
                    nc.scalar.copy(vexp1[:], vrep_ps[:])
                    return vexp1

                vexp1 = squash_and_bcast(s0_ps, 1.0 / O, last=False)

                # ---------------- rounds 1, 2 ----------------
                with (
                    tc.tile_pool(name="rnd", bufs=2) as rp,
                    tc.tile_pool(name="rnd2", bufs=3) as rp2,
                    tc.tile_pool(name="vex8", bufs=1) as v8p,
                ):
                    for rnd in (1, 2):
                        # replicate v over the t axis once per round so the
                        # hot vu-mul is a plain tensor-tensor op (2x DVE mode)
                        vexp8 = v8p.tile([128, BATCH, FREE], bf16, tag="vexp8")
                        for j in range(BATCH):
                            nc.vector.tensor_copy(vexp8[:, j, :], vexp1[:])
                        s_ps = psum_s.tile([BL, FREE], f32, tag="s_ps")

                        # Software-pipelined over groups with a 2-group skew:
                        # engines run their queues in order, so emitting
                        # rz/c/cu right after exp would head-of-line block
                        # DVE on ACT. front = agreement + exp, mid = rz + c,
                        # back = cu + PE accumulation.
                        state = {}

                        def front(g):
                            tb = g * BATCH
                            u_sl = u16[:, tb:tb + BATCH, :]
                            vu = rp.tile([128, BATCH, FREE], bf16, tag="vu")
                            nc.vector.tensor_mul(vu[:], u_sl, vexp8[:])
                            # k-tree: in (k,o) layout the k halves are
                            # contiguous column blocks, all ops stay 3-D
                            t1 = rp.tile([128, BATCH, FREE // 2], bf16, tag="t1")
                            nc.vector.tensor_add(
                                t1[:], vu[:, :, 0:256], vu[:, :, 256:512])
                            t2 = rp.tile([128, BATCH, FREE // 4], bf16, tag="t2")
                            nc.vector.tensor_add(
                                t2[:], t1[:, :, 0:128], t1[:, :, 128:256])
                            t3 = rp2.tile([128, BATCH, FREE // 8], bf16, tag="t3")
                            lgs = bL[:, tb:tb + BATCH, :]
                            nc.vector.tensor_add(
                                t3[:], t2[:, :, 0:64], t2[:, :, 64:128])
                            if rnd == 1:
                                # b1 = agreement (b0 == 0)
                                nc.vector.tensor_add(
                                    lgs, t3[:, :, 0:32], t3[:, :, 32:64])
                            else:
                                agr = rp2.tile([128, BATCH, O], bf16, tag="agr")
                                nc.vector.tensor_add(
                                    agr[:], t3[:, :, 0:32], t3[:, :, 32:64])
                                # b2 = b1 + agreement (in place)
                                nc.vector.tensor_add(lgs, agr[:], lgs)
                            # per-tile exp on ACT, accumulator supplies z
                            e = rp2.tile([128, BATCH, O], bf16, tag="e")
                            z = rp2.tile([128, BATCH], f32, tag="z")
                            for j in range(BATCH):
                                nc.scalar.activation(
                                    e[:, j, :], lgs[:, j, :], ACTF.Exp,
                                    accum_out=z[:, j:j + 1])
                            state[g] = (u_sl, e, z)

                        def mid(g):
                            u_sl, e, z = state[g]
                            rz = rp2.tile([128, BATCH], f32, tag="rz")
                            nc.vector.reciprocal(rz[:], z[:])
                            c = rp2.tile([128, BATCH, O], bf16, tag="c")
                            for j in range(BATCH):
                                nc.scalar.activation(
                                    c[:, j, :], e[:, j, :], ACTF.Copy,
                                    scale=rz[:, j:j + 1])
                            state[g] = (u_sl, c)

                        def back(g):
                            u_sl, c = state.pop(g)
                            cu = rp.tile([128, BATCH, K, O], bf16, tag="cu")
                            for j in range(BATCH):
                                nc.vector.tensor_mul(
                                    cu[:, j],
                                    u_sl[:, j].rearrange("p (k o) -> p k o", o=O),
                                    c[:, j].unsqueeze(1).broadcast_to(
                                        [128, K, O]))
                            # s += sum_i cu  (PE partition reduce via ones)
                            for j in range(BATCH):
                                t = g * BATCH + j
                                nc.tensor.matmul(
                                    s_ps[:], ones_sb[:],
                                    cu[:, j, :, :].rearrange("p k o -> p (k o)"),
                                    start=(t == 0), stop=(t == T - 1))

                        for g in range(NGRP + 2):
                            if g < NGRP:
                                front(g)
                            if 1 <= g < NGRP + 1:
                                mid(g - 1)
                            if g >= 2:
                                back(g - 2)
                        vexp1 = squash_and_bcast(s_ps, 1.0, last=(rnd == 2))
    nc.finalize()
    return nc


def _host_prep():
    """Core-independent input prep pieces."""
    ones = np.zeros((128, BL), dtype=BF16)
    for g in range(G):
        for b in range(BL):
            ones[g * 8 + b, b] = 1
    onest = np.ascontiguousarray(ones.T)
    return ones, onest


def kernel(x: np.ndarray, W: np.ndarray) -> np.ndarray:
    from concourse import bass_utils

    if "nc" not in _CACHE:
        _CACHE["nc"] = _build_bass()
        _CACHE["ones"], _CACHE["onest"] = _host_prep()
    nc = _CACHE["nc"]

    # W -> [(g,d), t, (k,o)] : w[g*8+d, t, k*32+o] = W[t*16+g, o, d, k]
    wr = np.ascontiguousarray(
        (W.reshape(T, G, O, D, K).transpose(0, 1, 3, 4, 2)
         .reshape(T, 128, FREE)).transpose(1, 0, 2)).astype(BF16)
    in_maps = []
    for c in range(NC_N):
        xl = x[c * BL:(c + 1) * BL]  # [8, 2048, 8]
        # xt[g*8+d, t, b] = xl[b, t*16+g, d]
        xt = np.ascontiguousarray(
            xl.reshape(BL, T, G, D).transpose(2, 3, 1, 0).reshape(128, T, BL)
        ).astype(BF16)
        xblk = np.zeros((128, T, 128), dtype=BF16)
        for g in range(G):
            xblk[g * 8:(g + 1) * 8, :, g * 8:(g + 1) * 8] = xt[g * 8:(g + 1) * 8]
        in_maps.append({"w": wr, "xt": xt, "xblk": xblk, "ones": _CACHE["ones"],
                        "onest": _CACHE["onest"]})

    _CACHE["in_maps"] = in_maps
    res = bass_utils.run_bass_kernel_spmd(nc, in_maps, core_ids=list(range(NC_N)))
    out = np.empty((B, O, K), np.float32)
    for c in range(NC_N):
        v = res.results[c]["out"].reshape(BL, K, O)  # (k,o) cols
        out[c * BL:(c + 1) * BL] = v.transpose(0, 2, 1)
    return out
